# revision 35
# baseline (speedup 1.0000x reference)
"""Trainium2 Bass kernel for masked multi-head attention (B=8, S=1024, HID=1024, NH=16).

Computation (matches the torch/jax reference):
    q = query @ Wk.T + bk ; k = key @ Wk.T + bk ; v = value @ Wv.T + bv
    per head: scores = q k^T / 8, masked softmax over keys (mask zeroes masked
    positions), out = probs @ v.

Sharding: data-parallel over batch - batch element b runs on NeuronCore b.

v2 design (vs the 324us v1): everything in bf16 on the matmul paths (fp32 psum),
PE tile-position packing so head pairs run concurrently, and fine-grained
interleaving of projection matmuls into the attention phase so the ACT-engine
exp stream hides under PE work.

  - inputs host-compacted to unmasked key positions (padded to KB*128), all
    matmul operands bf16: halves HBM traffic and enables fast weight load.
  - scores: heads 2g (partitions 0-63) and 2g+1 (partitions 64-127) of the
    same output-column block are row-group-tiled: two concurrent K=64 matmuls
    (tile_position (0,0) / (64,0)) into one [128,1024] psum -> 2x.
  - softmax: one ACT exp pass per (pair, kb, seg) over [128,1024] psum with
    per-partition mask bias; pad rows get -1e30 so they contribute exactly 0.
  - PV: col-group tiled pair - head 2g -> psum rows 0-63, head 2g+1 -> rows
    64-127 of one [128,1024] psum (tile_position (0,0)/(0,64)), 2x.
  - denominators: four M=1 col-tiles (partitions 0/32/64/96 of ONE psum bank)
    accumulate ones.T @ P^T per (head, seg) across kb.
  - normalize: reciprocal_approx_fast on the denom bank, K=1 broadcast
    matmuls (col-tiled 0-63/64-127), one fused [128,512] multiply per seg.
  - Q/K projections for pair g+1 are emitted inside pair g's attention steps
    (one [128,512] psum seg at a time from the shared scores pool) so the PE
    never idles while ACT drains exps.

PSUM budget (8 banks): scores pool 2x[128,1024]=4, PV pair [128,1024]=2,
denom pool 2x[97/128,512]=2.
"""

import os
import sys
from collections import deque
from contextlib import ExitStack

for _p in ("/opt/trn_rl_repo", "/root/.axon_site/_ro/trn_rl_repo"):
    if os.path.isdir(_p) and _p not in sys.path:
        sys.path.insert(0, _p)

import numpy as np
import ml_dtypes

from concourse import bacc, mybir, tile
from concourse.bass_utils import run_bass_kernel_spmd

B, S, HID, NH = 8, 1024, 1024, 16
HD = HID // NH  # 64
P = 128
JC = HID // P  # 8 contraction chunks for the projections
OB = HID // P  # 8 output-column blocks (head pairs)
NEG = -1.0e30

F32 = mybir.dt.float32
BF16 = mybir.dt.bfloat16
AF = mybir.ActivationFunctionType
NPBF16 = ml_dtypes.bfloat16

TRACE = os.environ.get("MHA_TRACE", "0") == "1"

_CACHE: dict = {}


def _ensure_axon_ntff_hook():
    """The agent image's antenv lacks axon_hooks; rebuild it from trn_boot's
    ctypes NTFF driver so trace=True can produce per-core profiles."""
    try:
        import antenv.axon_hooks  # noqa: F401

        return
    except ImportError:
        pass
    try:
        import types

        import antenv
        from trn_agent_boot.trn_boot import _ntff_profile_via_ctypes

        m = types.ModuleType("antenv.axon_hooks")
        m._hook = _ntff_profile_via_ctypes("/opt/axon/libaxon_pjrt.so")
        m.get_axon_ntff_profile_hook = lambda: m._hook
        m.set_axon_ntff_profile_hook = lambda h: setattr(m, "_hook", h)
        sys.modules["antenv.axon_hooks"] = m
        antenv.axon_hooks = m
    except Exception as e:  # pragma: no cover
        print(f"ntff hook shim unavailable: {e}", file=sys.stderr)


def _build(KB: int):
    """Build the SPMD program for compacted key length KC = KB*128."""
    KC = KB * P
    KSEGS = [(a, min(a + 512, KC)) for a in range(0, KC, 512)]
    nc = bacc.Bacc("TRN2", target_bir_lowering=False, debug=False)
    names = {}

    with tile.TileContext(nc) as tc, ExitStack() as ctx:
        dram = ctx.enter_context(tc.tile_pool(name="dram", bufs=1, space="DRAM"))

        def din(nm, shape, dt=BF16):
            t = dram.tile(shape, dt, kind="ExternalInput", name=nm, uniquify=False)
            names[nm] = t.name
            return t

        qT_d = din("qT", [HID, S])
        kT_d = din("kT", [HID, KC])
        vT_d = din("vT", [HID, KC])
        WkT_d = din("WkT", [HID, HID])
        WvT_d = din("WvT", [HID, HID])
        bkc_d = din("bkc", [P, OB], F32)
        bvb_d = din("bvb", [P, HID], F32)
        mkc_d = din("mkc", [P, KB], F32)
        sel_d = din("sel", [2, P])  # row0: ones cols 0-63; row1: ones 64-127
        outT_d = dram.tile(
            [HID, S], BF16, kind="ExternalOutput", name="outT", uniquify=False
        )
        names["out"] = outT_d.name

        res = ctx.enter_context(tc.tile_pool(name="res", bufs=1))
        QT = res.tile([P, OB, S], BF16, tag="QT")      # Q^T  [o, g, s]
        KT = res.tile([P, OB, KC], BF16, tag="KT")     # K^T  [o, g, k]
        Vx = res.tile([P, KB, HID], BF16, tag="Vx")    # V    [k, kb, h*64+d]
        bkc = res.tile([P, OB], F32, tag="bkc")
        bvb = res.tile([P, HID], F32, tag="bvb")
        mkc = res.tile([P, KB], F32, tag="mkc")
        sel = res.tile([2, P], BF16, tag="sel")
        onec = res.tile([P, 1], BF16, tag="onec")      # ones column (denoms)

        # persistent input tiles (qT/kT/WkT live until the last projection)
        qTt = res.tile([P, JC, S], BF16, tag="qTt")
        kTt = res.tile([P, JC, KC], BF16, tag="kTt")
        WkTt = res.tile([P, JC, HID], BF16, tag="WkTt")

        # ---- single PSUM pools for the whole kernel (8 banks total) -------
        # psS 2x[128,1024]=4, psPV [128,1024]=2, psD [128,512]=1, psX [128,512]=1
        psS = ctx.enter_context(tc.tile_pool(name="psS", bufs=2, space="PSUM"))
        psPV = ctx.enter_context(tc.tile_pool(name="psPV", bufs=1, space="PSUM"))
        psD = ctx.enter_context(tc.tile_pool(name="psD", bufs=1, space="PSUM"))
        psX = ctx.enter_context(tc.tile_pool(name="psX", bufs=1, space="PSUM"))

        ptp = ctx.enter_context(tc.tile_pool(name="ptp", bufs=4))
        oup = ctx.enter_context(tc.tile_pool(name="oup", bufs=2))
        onp = ctx.enter_context(tc.tile_pool(name="onp", bufs=2))
        rcp = ctx.enter_context(tc.tile_pool(name="rcp", bufs=2))
        rcrp = ctx.enter_context(tc.tile_pool(name="rcrp", bufs=2))

        # ---- PE warm-up: dummy matmuls with no data deps keep the PE busy
        # (and the HAM clock-gate at 8/8) while the input DMAs stream in.
        # Bursts are interleaved into V-proj sb0's DMA-paced chunk loop.
        wu = res.tile([P, 512], BF16, tag="wu")
        nc.gpsimd.memset(wu[:], 0.0)
        nc.gpsimd.memset(onec[:], 1.0)
        wu_sink = dram.tile(
            [1, 1], F32, kind="ExternalOutput", name="wu_sink", uniquify=False
        )
        wps = psX.tile([P, 512], F32, tag="X", name="wu_ps")

        def emit_warmup(n):
            for i in range(n):
                nc.tensor.matmul(
                    wps[:], wu[:, 0:P], wu[:], start=(i == 0), stop=(i == n - 1)
                )

        emit_warmup(8)

        # ---- input DMAs (small consts first, then V, K, W, Q chunks) ------
        nc.gpsimd.dma_start(bkc[:], bkc_d[:])
        nc.gpsimd.dma_start(bvb[:], bvb_d[:])
        nc.gpsimd.dma_start(mkc[:], mkc_d[:])
        nc.gpsimd.dma_start(sel[:], sel_d[:])
        # two DGE queues at ~156 GB/s each saturate the per-core HBM share;
        # a third queue only degrades per-queue rates.

        with tc.tile_pool(name="pv_in", bufs=1) as pvin:
            vTt = pvin.tile([P, JC, KC], BF16, tag="vTt")
            WvTt = pvin.tile([P, JC, HID], BF16, tag="WvTt")
            # DMA order: (vT, WvT) feed V-proj first, then (qT, WkT) for the
            # ob0 projections, kT last. DMA descriptor issue costs ~750ns on
            # the engine queue and completions serialize on a small semaphore
            # ring, so use few BIG strided descriptors (one per array half)
            # round-robined across all three DGE queues.
            for c in range(JC):
                nc.sync.dma_start(vTt[:, c, :], vT_d[c * P : (c + 1) * P, :])
                nc.scalar.dma_start(WvTt[:, c, :], WvT_d[c * P : (c + 1) * P, :])
            for c in range(JC):
                nc.sync.dma_start(qTt[:, c, :], qT_d[c * P : (c + 1) * P, :])
                nc.scalar.dma_start(WkTt[:, c, :], WkT_d[c * P : (c + 1) * P, :])
            for c in range(JC):
                nc.sync.dma_start(kTt[:, c, :], kT_d[c * P : (c + 1) * P, :])

            # ---- Q/K projection segment emitter --------------------------
            def emit_proj_seg(ob, which, a, b, pool):
                n = b - a
                src = qTt if which == "q" else kTt
                dst = QT if which == "q" else KT
                pp = pool.tile([P, n], F32, tag=pool_tag[id(pool)],
                               name=f"ps{which}{ob}_{a}")
                for c in range(JC):
                    nc.tensor.matmul(
                        pp[:], WkTt[:, c, ob * P : (ob + 1) * P], src[:, c, a:b],
                        start=(c == 0), stop=(c == JC - 1),
                    )
                nc.vector.tensor_scalar_add(
                    dst[:, ob, a:b], pp[:], bkc[:, ob : ob + 1]
                )

            pool_tag = {id(psS): "S", id(psX): "X"}

            # ---- phase V (V = value @ Wv^T + bv), then QK ob0 --------------
            # sb0 is DMA-paced (one chunk pair per ~1.1us): fill the PE gaps
            # with warm-up bursts so HAM stays at 8/8.
            for sb in range(KB):
                ps = psS.tile([P, S], F32, tag="S", name=f"psv{sb}")
                for c in range(JC):
                    if sb == 0:
                        emit_warmup(2)
                    lhsT = vTt[:, c, sb * P : (sb + 1) * P]
                    for a, b in ((0, 512), (512, 1024)):
                        nc.tensor.matmul(
                            ps[:, a:b], lhsT, WvTt[:, c, a:b],
                            start=(c == 0), stop=(c == JC - 1),
                        )
                nc.vector.tensor_add(Vx[:, sb, :], ps[:], bvb[:])
                emit_warmup(2)
            for w, a, b in (
                [("q", 0, 512), ("q", 512, 1024)] + [("k", a, b) for a, b in KSEGS]
            ):
                emit_warmup(4)
                emit_proj_seg(0, w, a, b, psS)
            wu_sb = res.tile([1, 1], F32, tag="wu_sb")
            nc.vector.tensor_copy(wu_sb[:], wps[0:1, 0:1])
            nc.sync.dma_start(wu_sink[:], wu_sb[:])

        # ---- interleaved main loop: attention pair g + projections g+1 ----
        # proj schedule: 4 segs of ob g+1 at the first four steps of pair g
        # (they fill the PE at the pair boundary where PV/den are not ready)
        proj_sched = {}
        for g in range(OB - 1):
            segs = [("q", 0, 512), ("q", 512, 1024)] + [
                ("k", a, b) for a, b in KSEGS
            ]
            spots = [(0, 0), (0, 1), (1, 0), (1, 1), (2, 0), (2, 1)]
            for i, sgd in enumerate(segs):
                proj_sched[(g,) + spots[i]] = (g + 1,) + sgd

        pair_state = {}  # g -> dict with psum/sbuf tiles of that pair
        pend_pv = deque()  # steps whose PV/denoms haven't been emitted yet

        def emit_pv_den(g, kb, seg, PT):
            st = pair_state[g]
            pvp, D = st["pvp"], st["D"]
            a = seg * 512
            h0, h1 = 2 * g, 2 * g + 1
            first, last = kb == 0, kb == KB - 1
            # PV col-pair: head h0 -> rows 0-63, h1 -> rows 64-127
            nc.tensor.matmul(
                pvp[0:HD, a : a + 512], Vx[:, kb, h0 * HD : (h0 + 1) * HD],
                PT[:, 0:512], start=first, stop=last, skip_group_check=True,
            )
            nc.tensor.matmul(
                pvp[HD:P, a : a + 512], Vx[:, kb, h1 * HD : (h1 + 1) * HD],
                PT[:, 512:1024], start=first, stop=last, skip_group_check=True,
            )
            # denominators: 4 M=1 col-tiles in one bank (rows 0/32/64/96)
            r = seg * 64
            nc.tensor.matmul(
                D[r : r + 1, :], onec[:], PT[:, 0:512],
                start=first, stop=last, skip_group_check=True,
                tile_position=(0, r),
            )
            nc.tensor.matmul(
                D[r + 32 : r + 33, :], onec[:], PT[:, 512:1024],
                start=first, stop=last, skip_group_check=True,
                tile_position=(0, r + 32),
            )

        def emit_passB(g, kb, seg):
            """Tail of pair g-1, spread across early steps of pair g."""
            gp = g - 1
            if gp < 0 or gp not in pair_state:
                return
            st = pair_state[gp]
            if (kb, seg) == (1, 0):
                # rc copy first: frees the D bank for pair g's denominators
                st["rc"] = rcp.tile([97, 512], F32, tag="rc", name=f"rc{gp}")
                nc.vector.tensor_copy(st["rc"][:], st["D"][0:97, :])
                st["Ou"] = oup.tile([P, S], F32, tag="Ou", name=f"Ou{gp}")
                nc.vector.tensor_copy(st["Ou"][:], st["pvp"][:])
                nc.vector.reciprocal_approx_fast(st["rc"][:], st["rc"][:])
            elif (kb, seg) == (1, 1):
                r0 = rcrp.tile([1, S], BF16, tag="rcr0", name=f"rcr0_{gp}")
                r1 = rcrp.tile([1, S], BF16, tag="rcr1", name=f"rcr1_{gp}")
                rc = st["rc"]
                nc.vector.tensor_copy(r0[:, 0:512], rc[0:1, :])
                nc.vector.tensor_copy(r0[:, 512:1024], rc[64:65, :])
                nc.vector.tensor_copy(r1[:, 0:512], rc[32:33, :])
                nc.vector.tensor_copy(r1[:, 512:1024], rc[96:97, :])
                st["r0"], st["r1"] = r0, r1
                st["On"] = onp.tile([P, S], BF16, tag="On", name=f"On{gp}")
            elif (kb, seg) in ((2, 0), (2, 1)):
                sg = 0 if (kb, seg) == (2, 0) else 1
                a = sg * 512
                bc = psX.tile([P, 512], F32, tag="X", name=f"bc{gp}_{sg}")
                nc.tensor.matmul(
                    bc[0:HD, :], sel[0:1, 0:HD], st["r0"][:, a : a + 512],
                    start=True, stop=True, skip_group_check=True,
                )
                nc.tensor.matmul(
                    bc[HD:P, :], sel[0:1, 0:HD], st["r1"][:, a : a + 512],
                    start=True, stop=True, skip_group_check=True,
                )
                nc.vector.tensor_mul(
                    st["On"][:, a : a + 512], st["Ou"][:, a : a + 512], bc[:]
                )
            elif (kb, seg) == (3, 0):
                eng = nc.gpsimd if gp % 2 == 0 else nc.sync
                eng.dma_start(outT_d[gp * P : (gp + 1) * P, :], st["On"][:])
                del pair_state[gp]

        for g in range(OB):
            pair_state[g] = {
                "pvp": psPV.tile([P, S], F32, tag="PV", name=f"pv{g}"),
                "D": psD.tile([P, 512], F32, tag="D", name=f"D{g}"),
            }
            for kb in range(KB):
                for seg in range(2):
                    pj = proj_sched.get((g, kb, seg))
                    if pj is not None:
                        emit_proj_seg(pj[0], pj[1], pj[2], pj[3], psX)
                    emit_passB(g, kb, seg)
                    if len(pend_pv) >= 2:
                        emit_pv_den(*pend_pv.popleft())
                    # scores pair: rows 0-1 (head 2g) / rows 2-3 (head 2g+1)
                    Sps = psS.tile([P, S], F32, tag="S", name=f"S{g}_{kb}_{seg}")
                    a = seg * 512
                    kbs = slice(kb * P, (kb + 1) * P)
                    nc.tensor.matmul(
                        Sps[:, 0:512], KT[0:HD, g, kbs], QT[0:HD, g, a : a + 512],
                        start=True, stop=True,
                    )
                    nc.tensor.matmul(
                        Sps[:, 512:1024], KT[HD:P, g, kbs], QT[HD:P, g, a : a + 512],
                        start=True, stop=True,
                    )
                    PT = ptp.tile([P, S], BF16, tag="PT", name=f"PT{g}_{kb}_{seg}")
                    nc.scalar.activation(
                        PT[:], Sps[:], AF.Exp, bias=mkc[:, kb : kb + 1], scale=0.125
                    )
                    pend_pv.append((g, kb, seg, PT))

        # drain: last PV/den + pass-B of the last pair
        while pend_pv:
            emit_pv_den(*pend_pv.popleft())
        for kb, seg in ((1, 0), (1, 1), (2, 0), (2, 1), (3, 0)):
            emit_passB(OB, kb, seg)

    nc.compile()
    return nc, names


def _prep(query, key, value, attention_mask, Wk, bk, Wv, bv):
    """Host-side sharding + layout prep. Returns (KB, in_maps, empty_batches)."""
    query = np.asarray(query, dtype=np.float32)
    key = np.asarray(key, dtype=np.float32)
    value = np.asarray(value, dtype=np.float32)
    mask = np.asarray(attention_mask).reshape(B, S) != 0
    Wk = np.asarray(Wk, dtype=np.float32)
    bk = np.asarray(bk, dtype=np.float32)
    Wv = np.asarray(Wv, dtype=np.float32)
    bv = np.asarray(bv, dtype=np.float32)

    idxs, counts = [], []
    for b in range(B):
        ix = np.flatnonzero(mask[b])
        idxs.append(ix)
        counts.append(len(ix))
    KC = max(int(np.ceil(max(max(counts), 1) / P)) * P, P)
    KB = KC // P

    WkT = np.ascontiguousarray(Wk.T.astype(NPBF16))
    WvT = np.ascontiguousarray(Wv.T.astype(NPBF16))
    bkc = np.ascontiguousarray(bk.reshape(OB, P).T)         # [128, 8]
    bvb = np.ascontiguousarray(np.broadcast_to(bv, (P, HID)))
    sel = np.zeros((2, P), dtype=NPBF16)
    sel[0, 0:HD] = 1
    sel[1, HD:P] = 1

    in_maps = []
    empty = []
    for b in range(B):
        n = counts[b]
        if n == 0:
            empty.append(b)
        ix = idxs[b] if n > 0 else np.array([0])
        pad = np.concatenate([ix, np.full(KC - len(ix), ix[0], dtype=ix.dtype)])
        mb = np.zeros(KC, dtype=np.float32)
        mb[n:] = NEG
        in_maps.append(
            {
                "qT": np.ascontiguousarray(query[b].T.astype(NPBF16)),
                "kT": np.ascontiguousarray(key[b].T[:, pad].astype(NPBF16)),
                "vT": np.ascontiguousarray(value[b].T[:, pad].astype(NPBF16)),
                "WkT": WkT,
                "WvT": WvT,
                "bkc": bkc,
                "bvb": bvb,
                "mkc": np.ascontiguousarray(mb.reshape(KB, P).T),
                "sel": sel,
            }
        )
    return KB, in_maps, empty


def kernel(key, value, query, attention_mask, Wk, bk, Wv, bv):
    KB, in_maps, empty = _prep(query, key, value, attention_mask, Wk, bk, Wv, bv)

    if KB not in _CACHE:
        _CACHE[KB] = _build(KB)
    nc, names = _CACHE[KB]

    mapped = [{names[k]: v for k, v in m.items()} for m in in_maps]
    if TRACE:
        _ensure_axon_ntff_hook()
    res = run_bass_kernel_spmd(nc, mapped, list(range(B)), trace=TRACE)
    if TRACE and res.exec_time_ns is not None:
        print(f"HW exec time: {res.exec_time_ns} ns")

    out = np.empty((B, S, HID), dtype=np.float32)
    for b in range(B):
        out[b] = np.asarray(res.results[b][names["out"]], dtype=np.float32).T
    for b in empty:
        out[b] = 0.0
    return out
